# revision 1
# baseline (speedup 1.0000x reference)
"""GQA attention kernel for 8 trn2 NeuronCores (Bass/Tile, SPMD).

Problem: X[2,2048,2048] fp32, Wq[2048,2048], Wk/Wv[2048,512], Wo[2048,2048].
  q/k/v proj -> GQA attention (32 Q heads, 8 KV heads, head_dim 64, no mask)
  -> out proj.

Sharding (8 cores): core c handles batch b=c//4 and query heads
[8j, 8j+8) with KV heads {2j, 2j+1} where j=c%4.  Within a core, heads are
processed in pairs (p, p+4) so that pair p puts head p (KV head 2j) on
partitions 0-63 and head p+4 (KV head 2j+1) on partitions 64-127.

Everything runs in a transposed layout so all matmul contractions sit on
the partition axis:
  proj:    Q^T = Wq_c^T X^T   (lhsT=Wq chunks, rhs=X^T chunks)  [fp32r]
  scores:  S^T = K Q^T        (row-packed pair of K=64 matmuls)  [fp32r]
  softmax: P^T = exp(S^T/8) via ScalarE (pair-batched [128,1024] ops),
           rowsum via a fused ones-column in the PV lhsT
  PV:      O^T = [V|1]^T P^T  (M=65 matmuls, row 64 = rowsum)    [fp32r]
  norm:    O^T *= bcast(1/rowsum)  (K=1 fp32 matmul broadcast + DVE mul)
  AllGather O^T slices within each 4-core (same-batch) group
  oproj:   Y^T = Wo_c^T O^T_full  (column shard -> disjoint output slices)

Host side only reshapes/permutes/transposes inputs and concatenates the
8 disjoint output slices.
"""

import os
import sys
import types
from contextlib import ExitStack

import numpy as np

_HIDDEN = 2048
_SEQ = 2048
_BATCH = 2
_NH = 32  # query heads
_NKV = 8
_HD = 64  # head dim
_NCORES = 8

_KC = _HIDDEN // 128  # 16 contraction chunks for proj/oproj
_NT = _SEQ // 512  # 4 seq tiles of 512
_ST = _SEQ // 128  # 16 key chunks of 128
_NPAIR = 4  # head pairs per core

_ORDER = [0, 4, 1, 5, 2, 6, 3, 7]  # within-core local head order (pairing)


def _install_ntff_hook():
    """antenv in this image lacks axon_hooks; synthesize it so the axon
    NTFF profiling path works when tracing is requested."""
    try:
        import antenv

        try:
            from antenv import axon_hooks  # noqa: F401

            return
        except ImportError:
            pass
        mod = types.ModuleType("antenv.axon_hooks")
        mod._hook = None
        mod.set_axon_ntff_profile_hook = lambda h: setattr(mod, "_hook", h)
        mod.get_axon_ntff_profile_hook = lambda: mod._hook
        sys.modules["antenv.axon_hooks"] = mod
        antenv.axon_hooks = mod
        from trn_agent_boot.trn_boot import _ntff_profile_via_ctypes

        so = "/opt/axon/libaxon_pjrt.so"
        if os.path.exists(so):
            mod.set_axon_ntff_profile_hook(_ntff_profile_via_ctypes(so))
    except Exception:
        pass


_install_ntff_hook()

import concourse.bass as bass  # noqa: E402
import concourse.tile as tile  # noqa: E402
from concourse import bacc, bass_utils, mybir  # noqa: E402
from concourse.bass_utils import run_bass_kernel_spmd  # noqa: E402
from concourse.masks import make_identity  # noqa: E402

# no S3 bucket in this container; keep trace artifacts local
bass_utils.upload_artifacts = lambda tmpdir: tmpdir

F32 = mybir.dt.float32
F32R = mybir.dt.float32r

_nc_cache = None
_last_results = None


def _build():
    nc = bacc.Bacc("TRN2", target_bir_lowering=False, debug=False, num_devices=8)

    xt_d = nc.declare_dram_parameter("xt", [_HIDDEN, _SEQ], F32, isOutput=False)
    wq_d = nc.declare_dram_parameter("wq", [_HIDDEN, 512], F32, isOutput=False)
    wk_d = nc.declare_dram_parameter("wk", [_HIDDEN, 128], F32, isOutput=False)
    wv_d = nc.declare_dram_parameter("wv", [_HIDDEN, 128], F32, isOutput=False)
    wo_d = nc.declare_dram_parameter("wo", [_HIDDEN, 512], F32, isOutput=False)
    ones_d = nc.declare_dram_parameter("ones", [128, _ST], F32, isOutput=False)
    yt_d = nc.declare_dram_parameter("yt", [512, _SEQ], F32, isOutput=True)

    at_loc = nc.dram_tensor("at_loc", [512, _SEQ], F32)
    at_full = nc.dram_tensor("at_full", [_HIDDEN, _SEQ], F32)

    with (
        tile.TileContext(nc) as tc,
        ExitStack() as ctx,
        nc.allow_low_precision(reason="fp32r==fp32 bits; PE rounds to FP22 on read"),
    ):
        const = ctx.enter_context(tc.tile_pool(name="const", bufs=1))

        # ---- persistent SBUF tensors -------------------------------------
        wq_sb = const.tile([128, _KC, 512], F32R, tag="wq")
        wk_sb = const.tile([128, _KC, 128], F32R, tag="wk")
        wv_sb = const.tile([128, _KC, 128], F32R, tag="wv")
        wo_sb = const.tile([128, _KC, 512], F32R, tag="wo")
        nc.sync.dma_start(
            out=wq_sb, in_=wq_d[:, :].rearrange("(kc p) m -> p kc m", p=128).bitcast(F32R)
        )
        nc.sync.dma_start(
            out=wk_sb, in_=wk_d[:, :].rearrange("(kc p) m -> p kc m", p=128).bitcast(F32R)
        )
        nc.sync.dma_start(
            out=wv_sb, in_=wv_d[:, :].rearrange("(kc p) m -> p kc m", p=128).bitcast(F32R)
        )
        nc.sync.dma_start(
            out=wo_sb, in_=wo_d[:, :].rearrange("(kc p) m -> p kc m", p=128).bitcast(F32R)
        )

        ident = const.tile([128, 128], F32, tag="ident")
        make_identity(nc, ident)
        ones_row = const.tile([1, 64], F32, tag="ones_row")
        nc.vector.memset(ones_row, 1.0)

        qt_sb = [
            const.tile([128, _SEQ], F32R, tag=f"qt{p}", name=f"qt{p}")
            for p in range(_NPAIR)
        ]
        kt_sb = const.tile([128, _SEQ], F32R, tag="kt")
        vt_sb = const.tile([128, _SEQ], F32, tag="vt")
        # [V_A | ones | V_B | ones] per key chunk
        vone = const.tile([128, _ST, 130], F32R, tag="vone")

        # ---- phase 1: projections ---------------------------------------
        with (
            tc.tile_pool(name="xt_pool", bufs=4) as xt_pool,
            tc.tile_pool(name="proj_ps", bufs=1, space="PSUM") as proj_ps,
        ):
            for n in range(_NT):
                psq = [
                    proj_ps.tile([128, 512], F32, tag=f"psq{m}", name=f"psq{m}")
                    for m in range(4)
                ]
                psk = proj_ps.tile([128, 512], F32, tag="psk")
                psv = proj_ps.tile([128, 512], F32, tag="psv")
                for kc in range(_KC):
                    xt_t = xt_pool.tile([128, 512], F32R, tag="xt")
                    nc.sync.dma_start(
                        out=xt_t,
                        in_=xt_d[
                            kc * 128 : (kc + 1) * 128, n * 512 : (n + 1) * 512
                        ].bitcast(F32R),
                    )
                    st, sp = kc == 0, kc == _KC - 1
                    for m in range(4):
                        nc.tensor.matmul(
                            psq[m],
                            wq_sb[:, kc, m * 128 : (m + 1) * 128],
                            xt_t,
                            start=st,
                            stop=sp,
                        )
                    nc.tensor.matmul(psk, wk_sb[:, kc, :], xt_t, start=st, stop=sp)
                    nc.tensor.matmul(psv, wv_sb[:, kc, :], xt_t, start=st, stop=sp)
                nsl = slice(n * 512, (n + 1) * 512)
                for m in range(4):
                    nc.scalar.copy(qt_sb[m][:, nsl], psq[m])
                nc.scalar.copy(kt_sb[:, nsl], psk)
                nc.scalar.copy(vt_sb[:, nsl], psv)
                # transpose V^T chunks -> V natural, build [V|1] lhsT tiles
                for t in range(4):
                    sc = n * 4 + t
                    tp = proj_ps.tile([128, 128], F32, tag="tp")
                    nc.tensor.transpose(
                        tp, vt_sb[:, sc * 128 : (sc + 1) * 128], ident
                    )
                    nc.scalar.copy(vone[:, sc, 0:64], tp[:, 0:64])
                    nc.scalar.copy(vone[:, sc, 65:129], tp[:, 64:128])
            nc.sync.dma_start(out=vone[:, :, 64], in_=ones_d[:, :].bitcast(F32R))
            nc.sync.dma_start(out=vone[:, :, 129], in_=ones_d[:, :].bitcast(F32R))

        # ---- phase 2: attention -----------------------------------------
        with (
            tc.tile_pool(name="p_pool", bufs=3) as p_pool,
            tc.tile_pool(name="nrm_pool", bufs=2) as nrm_pool,
            tc.tile_pool(name="sc_ps", bufs=2, space="PSUM") as sc_ps,
            tc.tile_pool(name="o_ps", bufs=2, space="PSUM") as o_ps,
        ):
            for p in range(_NPAIR):
                qsl_A = slice(0, 64)
                qsl_B = slice(64, 128)
                for n in range(_NT):
                    nsl = slice(n * 512, (n + 1) * 512)
                    o_pair = o_ps.tile([128, 1024], F32, tag="o_pair")
                    for kt in range(_ST):
                        ksl = slice(kt * 128, (kt + 1) * 128)
                        s_pair = sc_ps.tile([128, 1024], F32, tag="s_pair")
                        nc.tensor.matmul(
                            s_pair[:, 0:512],
                            kt_sb[qsl_A, ksl],
                            qt_sb[p][qsl_A, nsl],
                            start=True,
                            stop=True,
                            tile_position=(0, 0),
                        )
                        nc.tensor.matmul(
                            s_pair[:, 512:1024],
                            kt_sb[qsl_B, ksl],
                            qt_sb[p][qsl_B, nsl],
                            start=True,
                            stop=True,
                            tile_position=(64, 0),
                        )
                        pp = p_pool.tile([128, 1024], F32R, tag="pp")
                        nc.scalar.activation(
                            pp, s_pair, mybir.ActivationFunctionType.Exp, scale=0.125
                        )
                        st, sp = kt == 0, kt == _ST - 1
                        nc.tensor.matmul(
                            o_pair[0:65, 0:512],
                            vone[:, kt, 0:65],
                            pp[:, 0:512],
                            start=st,
                            stop=sp,
                        )
                        nc.tensor.matmul(
                            o_pair[0:65, 512:1024],
                            vone[:, kt, 65:130],
                            pp[:, 512:1024],
                            start=st,
                            stop=sp,
                        )
                    # normalize: recip of rowsums, matmul-broadcast, multiply
                    rr = nrm_pool.tile([1, 1024], F32, tag="rr")
                    nc.vector.reciprocal(rr, o_pair[64:65, :])
                    rb_ps = sc_ps.tile([128, 1024], F32, tag="s_pair")
                    nc.tensor.matmul(
                        rb_ps[0:64, 0:512],
                        ones_row,
                        rr[0:1, 0:512],
                        start=True,
                        stop=True,
                    )
                    nc.tensor.matmul(
                        rb_ps[0:64, 512:1024],
                        ones_row,
                        rr[0:1, 512:1024],
                        start=True,
                        stop=True,
                    )
                    rb_sb = nrm_pool.tile([64, 1024], F32, tag="rb")
                    nc.vector.tensor_copy(rb_sb, rb_ps[0:64, :])
                    at_pair = nrm_pool.tile([64, 1024], F32, tag="at")
                    nc.vector.tensor_mul(at_pair, o_pair[0:64, :], rb_sb)
                    nc.sync.dma_start(
                        out=at_loc[p * 128 : p * 128 + 64, nsl],
                        in_=at_pair[:, 0:512],
                    )
                    nc.sync.dma_start(
                        out=at_loc[p * 128 + 64 : p * 128 + 128, nsl],
                        in_=at_pair[:, 512:1024],
                    )

        # ---- phase 3: AllGather within same-batch groups ----------------
        nc.gpsimd.collective_compute(
            "AllGather",
            mybir.AluOpType.bypass,
            replica_groups=[[0, 1, 2, 3], [4, 5, 6, 7]],
            ins=[at_loc[:, :]],
            outs=[at_full[:, :]],
        )

        # ---- phase 4: output projection (Y^T = Wo_c^T @ A^T_full) -------
        with (
            tc.tile_pool(name="ac_pool", bufs=4) as ac_pool,
            tc.tile_pool(name="y_pool", bufs=2) as y_pool,
            tc.tile_pool(name="y_ps", bufs=1, space="PSUM") as y_ps,
        ):
            for n in range(_NT):
                nsl = slice(n * 512, (n + 1) * 512)
                psy = [
                    y_ps.tile([128, 512], F32, tag=f"psy{m}", name=f"psy{m}")
                    for m in range(4)
                ]
                for kc in range(_KC):
                    ac_t = ac_pool.tile([128, 512], F32R, tag="ac")
                    nc.sync.dma_start(
                        out=ac_t,
                        in_=at_full[kc * 128 : (kc + 1) * 128, nsl].bitcast(F32R),
                    )
                    st, sp = kc == 0, kc == _KC - 1
                    for m in range(4):
                        nc.tensor.matmul(
                            psy[m],
                            wo_sb[:, kc, m * 128 : (m + 1) * 128],
                            ac_t,
                            start=st,
                            stop=sp,
                        )
                for m in range(4):
                    y_sb = y_pool.tile([128, 512], F32, tag="y")
                    nc.scalar.copy(y_sb, psy[m])
                    nc.sync.dma_start(
                        out=yt_d[m * 128 : (m + 1) * 128, nsl], in_=y_sb
                    )

    nc.compile()
    return nc


def kernel(X, Wq, Wk, Wv, Wo):
    global _nc_cache, _last_results
    X = np.ascontiguousarray(np.asarray(X, dtype=np.float32))
    Wq = np.asarray(Wq, dtype=np.float32)
    Wk = np.asarray(Wk, dtype=np.float32)
    Wv = np.asarray(Wv, dtype=np.float32)
    Wo = np.asarray(Wo, dtype=np.float32)

    if _nc_cache is None:
        _nc_cache = _build()
    nc = _nc_cache

    xts = [np.ascontiguousarray(X[b].T) for b in range(_BATCH)]
    perm_rows = []
    for j in range(4):
        for o in _ORDER:
            h = 8 * j + o
            perm_rows.extend(range(h * _HD, (h + 1) * _HD))
    wo_p = Wo[perm_rows, :]
    ones = np.ones((128, _ST), dtype=np.float32)

    in_maps = []
    for c in range(_NCORES):
        b, j = divmod(c, 4)
        qcols = []
        for o in _ORDER:
            h = 8 * j + o
            qcols.extend(range(h * _HD, (h + 1) * _HD))
        in_maps.append(
            {
                "xt": xts[b],
                "wq": np.ascontiguousarray(Wq[:, qcols]),
                "wk": np.ascontiguousarray(Wk[:, 2 * j * _HD : (2 * j + 2) * _HD]),
                "wv": np.ascontiguousarray(Wv[:, 2 * j * _HD : (2 * j + 2) * _HD]),
                "wo": np.ascontiguousarray(wo_p[:, 512 * j : 512 * (j + 1)]),
                "ones": ones,
            }
        )

    trace = bool(os.environ.get("KERNEL_TRACE"))
    res = run_bass_kernel_spmd(
        nc, in_maps, core_ids=list(range(_NCORES)), trace=trace
    )
    _last_results = res

    Y = np.empty((_BATCH, _SEQ, _HIDDEN), dtype=np.float32)
    for c in range(_NCORES):
        b, j = divmod(c, 4)
        Y[b][:, 512 * j : 512 * (j + 1)] = res.results[c]["yt"].T
    return Y



# revision 12
# speedup vs baseline: 1.4449x; 1.4449x over previous
"""GQA attention kernel for 8 trn2 NeuronCores (Bass/Tile, SPMD) — V2.

Problem: X[2,2048,2048] fp32, Wq[2048,2048], Wk/Wv[2048,512], Wo[2048,2048].
  q/k/v proj -> GQA attention (32 Q heads, 8 KV heads, head_dim 64, no mask)
  -> out proj.

V2 sharding: SEQUENCE-parallel attention. Core c handles batch b=c//4 and
query-sequence quarter j=c%4 (positions [512j, 512j+512)) for ALL 32 heads.
  - KV proj: each core computes K^T,V^T for its seq quarter only, then one
    small fp16 AllGather (1MB in / 4MB out per core) within the same-batch
    4-core group produces the full K^T/V^T.  The gather overlaps Q proj.
  - Attention + output proj then run entirely core-locally: no late
    collective, no DRAM roundtrip for the attention output.
Head packing: unit u=0..15 pairs q-heads (u, u+16) whose KV heads are
(u//4, u//4+4) — packed on partition halves so score/PV matmuls use the
full 128-partition array (baseline's pairing trick).

All streamed matmul operands are fp16 (same PE rate as fp32r, half the
DMA/SBUF), accumulation stays fp32 in PSUM.  Softmax normalization is kept
entirely off the PE critical path: rowsums come from a fused ones-column in
the PV lhsT; reciprocals use the fast custom-DVE op; the per-column
broadcast runs as a tiny fp32r matmul emitted one unit late so its DVE
dependency is long resolved by the time the PE reaches it.
"""

import os
import sys
import types
from contextlib import ExitStack

import numpy as np

_HIDDEN = 2048
_SEQ = 2048
_BATCH = 2
_NH = 32  # query heads
_NKV = 8
_HD = 64  # head dim
_NCORES = 8

_SQ = 512  # seq quarter per core
_KC = _HIDDEN // 128  # 16 contraction chunks
_NU = 16  # head-pair units per core (all 32 heads)
_ST = _SEQ // 128  # 16 key chunks of 128


def _install_ntff_hook():
    """antenv in this image lacks axon_hooks; synthesize it so the axon
    NTFF profiling path works when tracing is requested."""
    try:
        import antenv

        try:
            from antenv import axon_hooks  # noqa: F401

            return
        except ImportError:
            pass
        mod = types.ModuleType("antenv.axon_hooks")
        mod._hook = None
        mod.set_axon_ntff_profile_hook = lambda h: setattr(mod, "_hook", h)
        mod.get_axon_ntff_profile_hook = lambda: mod._hook
        sys.modules["antenv.axon_hooks"] = mod
        antenv.axon_hooks = mod
        from trn_agent_boot.trn_boot import _ntff_profile_via_ctypes

        so = "/opt/axon/libaxon_pjrt.so"
        if os.path.exists(so):
            mod.set_axon_ntff_profile_hook(_ntff_profile_via_ctypes(so))
    except Exception:
        pass


_install_ntff_hook()

import concourse.bass as bass  # noqa: E402
import concourse.tile as tile  # noqa: E402
from concourse import bacc, bass_utils, mybir  # noqa: E402
from concourse.bass_utils import run_bass_kernel_spmd  # noqa: E402

# no S3 bucket in this container; keep trace artifacts local
bass_utils.upload_artifacts = lambda tmpdir: tmpdir

F32 = mybir.dt.float32
F32R = mybir.dt.float32r
F16 = mybir.dt.float16

_nc_cache = None
_last_results = None


def _build():
    nc = bacc.Bacc("TRN2", target_bir_lowering=False, debug=False, num_devices=8)

    xt_d = nc.declare_dram_parameter("xt", [128, _KC, _SQ], F16, isOutput=False)
    wk_d = nc.declare_dram_parameter("wk", [128, _KC, 512], F16, isOutput=False)
    wv_d = nc.declare_dram_parameter("wv", [128, _KC, 512], F16, isOutput=False)
    wq_d = nc.declare_dram_parameter("wq", [_NU, 128, _KC, 128], F16, isOutput=False)
    wo_d = nc.declare_dram_parameter("wo", [16, 128, _KC, 128], F16, isOutput=False)
    yt_d = nc.declare_dram_parameter("yt", [_HIDDEN, _SQ], F32, isOutput=True)

    # K^T local [512, SQ] then V^T local [512, SQ]
    kv_loc = nc.dram_tensor("kv_loc", [1024, _SQ], F16)
    kv_full = nc.dram_tensor("kv_full", [4096, _SQ], F16)

    dbg = bool(os.environ.get("KERNEL_DEBUG"))
    if dbg:
        dbg_kt = nc.declare_dram_parameter("dbg_kt", [128, 4, _SEQ], F16, isOutput=True)
        dbg_vone = nc.declare_dram_parameter(
            "dbg_vone", [128, _ST, 4, 130], F16, isOutput=True
        )
        dbg_qt = nc.declare_dram_parameter(
            "dbg_qt", [128, _NU, _SQ], F16, isOutput=True
        )
        dbg_at = nc.declare_dram_parameter(
            "dbg_at", [128, _NU, _SQ], F16, isOutput=True
        )

    with (
        tile.TileContext(nc) as tc,
        ExitStack() as ctx,
        nc.allow_low_precision(reason="fp16 streams; fp32 PSUM accumulation"),
    ):
        const = ctx.enter_context(tc.tile_pool(name="const", bufs=1))

        # ---- persistent SBUF tensors -------------------------------------
        kt_sb = const.tile([128, 4, _SEQ], F16, tag="kt")  # gg-pair blocks
        vone = const.tile([128, _ST, 4, 130], F16, tag="vone")
        qt_sb = const.tile([128, _NU, _SQ], F16, tag="qt")
        at_sb = const.tile([128, _NU, _SQ], F16, tag="at")
        ones_f32 = const.tile([1, 64], F32, tag="ones_f32")
        nc.vector.memset(ones_f32, 1.0)
        ones_row = const.tile([1, 64], F32R, tag="ones_row")
        nc.scalar.copy(ones_row, ones_f32)
        nc.vector.memset(vone, 1.0)  # ones cols 64/129 survive the V copies

        # ---- phase 1: KV projection (own seq quarter) + gather ----------
        with tc.tile_pool(name="xt_pool", bufs=1) as xt_pool:
            xt_sb = xt_pool.tile([128, _KC, _SQ], F16, tag="xt")
            nc.sync.dma_start(out=xt_sb, in_=xt_d[:, :, :])

            with (
                tc.tile_pool(name="wkv_pool", bufs=1) as wkv_pool,
                tc.tile_pool(name="kv_ps", bufs=4, space="PSUM") as kv_ps,
                tc.tile_pool(name="kvo_pool", bufs=4) as kvo_pool,
            ):
                wk_sb = wkv_pool.tile([128, _KC, 512], F16, tag="wk")
                wv_sb = wkv_pool.tile([128, _KC, 512], F16, tag="wv")
                nc.sync.dma_start(out=wk_sb, in_=wk_d[:, :, :])
                nc.sync.dma_start(out=wv_sb, in_=wv_d[:, :, :])
                for gg in range(4):
                    msl = slice(gg * 128, (gg + 1) * 128)
                    psk = kv_ps.tile([128, _SQ], F32, tag="psk")
                    psv = kv_ps.tile([128, _SQ], F32, tag="psv")
                    for kc in range(_KC):
                        st, sp = kc == 0, kc == _KC - 1
                        nc.tensor.matmul(
                            psk, wk_sb[:, kc, msl], xt_sb[:, kc, :], start=st, stop=sp
                        )
                        nc.tensor.matmul(
                            psv, wv_sb[:, kc, msl], xt_sb[:, kc, :], start=st, stop=sp
                        )
                    ko = kvo_pool.tile([128, _SQ], F16, tag="ko")
                    vo = kvo_pool.tile([128, _SQ], F16, tag="vo")
                    nc.scalar.copy(ko, psk)
                    nc.scalar.copy(vo, psv)
                    nc.sync.dma_start(out=kv_loc[gg * 128 : (gg + 1) * 128, :], in_=ko)
                    nc.sync.dma_start(
                        out=kv_loc[512 + gg * 128 : 512 + (gg + 1) * 128, :], in_=vo
                    )

            # gather K/V across the same-batch group (overlaps Q proj below)
            nc.gpsimd.collective_compute(
                "AllGather",
                mybir.AluOpType.bypass,
                replica_groups=[[0, 1, 2, 3], [4, 5, 6, 7]],
                ins=[kv_loc[:, :]],
                outs=[kv_full[:, :]],
            )
            # unpack K^T into gg-pair blocks; seq on free axis
            with tc.tile_pool(name="vst_pool", bufs=4) as vst_pool:
                for jj in range(4):
                    jsl = slice(jj * _SQ, (jj + 1) * _SQ)
                    for gg in range(4):
                        nc.sync.dma_start(
                            out=kt_sb[:, gg, jsl],
                            in_=kv_full[
                                jj * 1024 + gg * 128 : jj * 1024 + (gg + 1) * 128, :
                            ],
                        )
                        # V natural via DMA-XBAR transpose (aligned staging,
                        # then copy around the fused ones columns)
                        r0 = jj * 1024 + 512 + gg * 128
                        for c in range(4):
                            sc = jj * 4 + c
                            csl = slice(c * 128, (c + 1) * 128)
                            vst = vst_pool.tile([128, 128], F16, tag="vst")
                            nc.sync.dma_start(
                                out=vst, in_=kv_full[r0 : r0 + 128, csl], transpose=True
                            )
                            nc.scalar.copy(vone[:, sc, gg, 0:64], vst[:, 0:64])
                            nc.scalar.copy(vone[:, sc, gg, 65:129], vst[:, 64:128])

            # ---- phase 2: Q projection (all 16 units) -------------------
            with (
                tc.tile_pool(name="wq_pool", bufs=3) as wq_pool,
                tc.tile_pool(name="q_ps", bufs=3, space="PSUM") as q_ps,
            ):
                for u in range(_NU):
                    wqt = wq_pool.tile([128, _KC, 128], F16, tag="wqt")
                    nc.sync.dma_start(out=wqt, in_=wq_d[u, :, :, :])
                    psq = q_ps.tile([128, _SQ], F32, tag="psq")
                    for kc in range(_KC):
                        nc.tensor.matmul(
                            psq,
                            wqt[:, kc, :],
                            xt_sb[:, kc, :],
                            start=kc == 0,
                            stop=kc == _KC - 1,
                        )
                    nc.scalar.copy(qt_sb[:, u, :], psq)

        # ---- phase 3: attention (per head-pair unit, fully local) -------
        with (
            tc.tile_pool(name="p_pool", bufs=3) as p_pool,
            tc.tile_pool(name="ou_pool", bufs=4) as ou_pool,
            tc.tile_pool(name="nrm_pool", bufs=3) as nrm_pool,
            tc.tile_pool(name="atm_pool", bufs=2) as atm_pool,
            tc.tile_pool(name="sc_ps", bufs=2, space="PSUM") as sc_ps,
            tc.tile_pool(name="o_ps", bufs=2, space="PSUM") as o_ps,
        ):
            pend = []  # (u, ou_tile, rowsum_tile) awaiting normalization

            def finalize():
                u, ou_u, rs_u = pend.pop(0)
                rc = nrm_pool.tile([1, 1024], F32, tag="rc")
                nc.vector.reciprocal_approx_fast(rc, rs_u)
                rcr = nrm_pool.tile([1, 1024], F32R, tag="rcr")
                nc.scalar.copy(rcr, rc)
                rb_ps = sc_ps.tile([128, 1024], F32, tag="s_pair")
                nc.tensor.matmul(
                    rb_ps[0:64, 0:512],
                    ones_row,
                    rcr[0:1, 0:512],
                    start=True,
                    stop=True,
                )
                nc.tensor.matmul(
                    rb_ps[0:64, 512:1024],
                    ones_row,
                    rcr[0:1, 512:1024],
                    start=True,
                    stop=True,
                )
                rb_sb = nrm_pool.tile([64, 1024], F16, tag="rb")
                nc.vector.tensor_copy(rb_sb, rb_ps[0:64, :])
                atm = atm_pool.tile([64, 1024], F16, tag="atm")
                nc.vector.tensor_mul(atm, ou_u, rb_sb)
                nc.sync.dma_start(out=at_sb[0:64, u, :], in_=atm[:, 0:512])
                nc.sync.dma_start(out=at_sb[64:128, u, :], in_=atm[:, 512:1024])

            for u in range(_NU):
                gg = u // 4
                o_pair = o_ps.tile([65, 1024], F32, tag="o_pair")
                for kt in range(_ST):
                    ksl = slice(kt * 128, (kt + 1) * 128)
                    s_pair = sc_ps.tile([128, 1024], F32, tag="s_pair")
                    nc.tensor.matmul(
                        s_pair[:, 0:512],
                        kt_sb[0:64, gg, ksl],
                        qt_sb[0:64, u, :],
                        start=True,
                        stop=True,
                        tile_position=(0, 0),
                    )
                    nc.tensor.matmul(
                        s_pair[:, 512:1024],
                        kt_sb[64:128, gg, ksl],
                        qt_sb[64:128, u, :],
                        start=True,
                        stop=True,
                        tile_position=(64, 0),
                    )
                    pp = p_pool.tile([128, 1024], F16, tag="pp")
                    nc.scalar.activation(
                        pp, s_pair, mybir.ActivationFunctionType.Exp, scale=0.125
                    )
                    st, sp = kt == 0, kt == _ST - 1
                    nc.tensor.matmul(
                        o_pair[0:65, 0:512],
                        vone[:, kt, gg, 0:65],
                        pp[:, 0:512],
                        start=st,
                        stop=sp,
                    )
                    nc.tensor.matmul(
                        o_pair[0:65, 512:1024],
                        vone[:, kt, gg, 65:130],
                        pp[:, 512:1024],
                        start=st,
                        stop=sp,
                    )
                    if kt == 2 and pend:
                        finalize()
                ou_u = ou_pool.tile([64, 1024], F16, tag="ou")
                nc.scalar.copy(ou_u, o_pair[0:64, :])
                rs_u = nrm_pool.tile([1, 1024], F32, tag="rs")
                nc.scalar.copy(rs_u, o_pair[64:65, :])
                pend.append((u, ou_u, rs_u))
            finalize()

        if dbg:
            nc.sync.dma_start(out=dbg_kt[:, :, :], in_=kt_sb)
            nc.sync.dma_start(out=dbg_vone[:, :, :, :], in_=vone)
            nc.sync.dma_start(out=dbg_qt[:, :, :], in_=qt_sb)
            nc.sync.dma_start(out=dbg_at[:, :, :], in_=at_sb)

        # ---- phase 4: output projection (own seq quarter, full output) --
        with (
            tc.tile_pool(name="wo_pool", bufs=3) as wo_pool,
            tc.tile_pool(name="y_pool", bufs=2) as y_pool,
            tc.tile_pool(name="y_ps", bufs=3, space="PSUM") as y_ps,
        ):
            for m in range(16):
                wot = wo_pool.tile([128, _KC, 128], F16, tag="wot")
                nc.sync.dma_start(out=wot, in_=wo_d[m, :, :, :])
                psy = y_ps.tile([128, _SQ], F32, tag="psy")
                for kc in range(_KC):
                    nc.tensor.matmul(
                        psy,
                        wot[:, kc, :],
                        at_sb[:, kc, :],
                        start=kc == 0,
                        stop=kc == _KC - 1,
                    )
                y_sb = y_pool.tile([128, _SQ], F32, tag="y")
                nc.scalar.copy(y_sb, psy)
                nc.sync.dma_start(out=yt_d[m * 128 : (m + 1) * 128, :], in_=y_sb)

    nc.compile()
    return nc


def kernel(X, Wq, Wk, Wv, Wo):
    global _nc_cache, _last_results
    X = np.asarray(X, dtype=np.float32)
    Wq = np.asarray(Wq, dtype=np.float32)
    Wk = np.asarray(Wk, dtype=np.float32)
    Wv = np.asarray(Wv, dtype=np.float32)
    Wo = np.asarray(Wo, dtype=np.float32)

    if _nc_cache is None:
        _nc_cache = _build()
    nc = _nc_cache

    # head-pair-unit permutation: unit u -> q heads (u, u+16)
    qcols = []
    for u in range(_NU):
        qcols.extend(range(u * _HD, (u + 1) * _HD))
        qcols.extend(range((u + 16) * _HD, (u + 17) * _HD))
    # kv-pair blocks: gg -> kv heads (gg, gg+4)
    kvcols = []
    for gg in range(4):
        kvcols.extend(range(gg * _HD, (gg + 1) * _HD))
        kvcols.extend(range((gg + 4) * _HD, (gg + 5) * _HD))

    wq_p = np.ascontiguousarray(
        Wq.astype(np.float16)[:, qcols]
        .reshape(_KC, 128, _NU, 128)
        .transpose(2, 1, 0, 3)
    )
    wk_p = np.ascontiguousarray(
        Wk.astype(np.float16)[:, kvcols].reshape(_KC, 128, 512).transpose(1, 0, 2)
    )
    wv_p = np.ascontiguousarray(
        Wv.astype(np.float16)[:, kvcols].reshape(_KC, 128, 512).transpose(1, 0, 2)
    )
    wo_p = np.ascontiguousarray(
        Wo.astype(np.float16)[qcols, :]
        .reshape(_KC, 128, 16, 128)
        .transpose(2, 1, 0, 3)
    )

    X16 = X.astype(np.float16)
    in_maps = []
    for c in range(_NCORES):
        b, j = divmod(c, 4)
        xt_q = np.ascontiguousarray(
            X16[b].T[:, j * _SQ : (j + 1) * _SQ]
            .reshape(_KC, 128, _SQ)
            .transpose(1, 0, 2)
        )
        in_maps.append(
            {"xt": xt_q, "wq": wq_p, "wk": wk_p, "wv": wv_p, "wo": wo_p}
        )

    trace = bool(os.environ.get("KERNEL_TRACE"))
    res = run_bass_kernel_spmd(
        nc, in_maps, core_ids=list(range(_NCORES)), trace=trace
    )
    _last_results = res

    Y = np.empty((_BATCH, _SEQ, _HIDDEN), dtype=np.float32)
    for c in range(_NCORES):
        b, j = divmod(c, 4)
        Y[b][j * _SQ : (j + 1) * _SQ, :] = res.results[c]["yt"].T
    return Y


# revision 17
# speedup vs baseline: 1.6177x; 1.1197x over previous
"""GQA attention kernel for 8 trn2 NeuronCores (Bass/Tile, SPMD) — V2.

Problem: X[2,2048,2048] fp32, Wq[2048,2048], Wk/Wv[2048,512], Wo[2048,2048].
  q/k/v proj -> GQA attention (32 Q heads, 8 KV heads, head_dim 64, no mask)
  -> out proj.

V2 sharding: SEQUENCE-parallel attention. Core c handles batch b=c//4 and
query-sequence quarter j=c%4 (positions [512j, 512j+512)) for ALL 32 heads.
  - KV proj: each core computes K^T,V^T for its seq quarter only, then one
    small fp16 AllGather (1MB in / 4MB out per core) within the same-batch
    4-core group produces the full K^T/V^T.  The gather overlaps Q proj.
  - Attention + output proj then run entirely core-locally: no late
    collective, no DRAM roundtrip for the attention output.
Head packing: unit u=0..15 pairs q-heads (u, u+16) whose KV heads are
(u//4, u//4+4) — packed on partition halves so score/PV matmuls use the
full 128-partition array (baseline's pairing trick).

All streamed matmul operands are fp16 (same PE rate as fp32r, half the
DMA/SBUF), accumulation stays fp32 in PSUM.  Softmax normalization is kept
entirely off the PE critical path: rowsums come from a fused ones-column in
the PV lhsT; reciprocals use the fast custom-DVE op; the per-column
broadcast runs as a tiny fp32r matmul emitted one unit late so its DVE
dependency is long resolved by the time the PE reaches it.
"""

import os
import sys
import types
from contextlib import ExitStack

import numpy as np

_HIDDEN = 2048
_SEQ = 2048
_BATCH = 2
_NH = 32  # query heads
_NKV = 8
_HD = 64  # head dim
_NCORES = 8

_SQ = 512  # seq quarter per core
_KC = _HIDDEN // 128  # 16 contraction chunks
_NU = 16  # head-pair units per core (all 32 heads)
_ST = _SEQ // 128  # 16 key chunks of 128


def _install_ntff_hook():
    """antenv in this image lacks axon_hooks; synthesize it so the axon
    NTFF profiling path works when tracing is requested."""
    try:
        import antenv

        try:
            from antenv import axon_hooks  # noqa: F401

            return
        except ImportError:
            pass
        mod = types.ModuleType("antenv.axon_hooks")
        mod._hook = None
        mod.set_axon_ntff_profile_hook = lambda h: setattr(mod, "_hook", h)
        mod.get_axon_ntff_profile_hook = lambda: mod._hook
        sys.modules["antenv.axon_hooks"] = mod
        antenv.axon_hooks = mod
        from trn_agent_boot.trn_boot import _ntff_profile_via_ctypes

        so = "/opt/axon/libaxon_pjrt.so"
        if os.path.exists(so):
            mod.set_axon_ntff_profile_hook(_ntff_profile_via_ctypes(so))
    except Exception:
        pass


_install_ntff_hook()

import concourse.bass as bass  # noqa: E402
import concourse.tile as tile  # noqa: E402
from concourse import bacc, bass_utils, mybir  # noqa: E402
from concourse.bass_utils import run_bass_kernel_spmd  # noqa: E402

# no S3 bucket in this container; keep trace artifacts local
bass_utils.upload_artifacts = lambda tmpdir: tmpdir

F32 = mybir.dt.float32
F32R = mybir.dt.float32r
F16 = mybir.dt.float16

_nc_cache = None
_last_results = None


def _build():
    nc = bacc.Bacc("TRN2", target_bir_lowering=False, debug=False, num_devices=8)

    xt_d = nc.declare_dram_parameter("xt", [128, _KC, _SQ], F16, isOutput=False)
    wk_d = nc.declare_dram_parameter("wk", [128, _KC, 512], F16, isOutput=False)
    wv_d = nc.declare_dram_parameter("wv", [128, _KC, 512], F16, isOutput=False)
    wq_d = nc.declare_dram_parameter("wq", [_NU, 128, _KC, 128], F16, isOutput=False)
    wo_d = nc.declare_dram_parameter("wo", [16, 128, _KC, 128], F16, isOutput=False)
    yt_d = nc.declare_dram_parameter("yt", [_HIDDEN, _SQ], F32, isOutput=True)

    # K^T local [512, SQ] then V^T local [512, SQ]
    kv_loc = nc.dram_tensor("kv_loc", [1024, _SQ], F16)
    kv_full = nc.dram_tensor("kv_full", [4096, _SQ], F16)

    dbg = bool(os.environ.get("KERNEL_DEBUG"))
    if dbg:
        dbg_kt = nc.declare_dram_parameter("dbg_kt", [128, 4, _SEQ], F16, isOutput=True)
        dbg_vone = nc.declare_dram_parameter(
            "dbg_vone", [128, _ST, 4, 130], F16, isOutput=True
        )
        dbg_qt = nc.declare_dram_parameter(
            "dbg_qt", [128, _NU, _SQ], F16, isOutput=True
        )
        dbg_at = nc.declare_dram_parameter(
            "dbg_at", [128, _NU, _SQ], F16, isOutput=True
        )

    with (
        tile.TileContext(nc) as tc,
        ExitStack() as ctx,
        nc.allow_low_precision(reason="fp16 streams; fp32 PSUM accumulation"),
    ):
        const = ctx.enter_context(tc.tile_pool(name="const", bufs=1))

        # ---- persistent SBUF tensors -------------------------------------
        kt_sb = const.tile([128, 4, _SEQ], F16, tag="kt")  # gg-pair blocks
        vone = const.tile([128, _ST, 4, 130], F16, tag="vone")
        qt_sb = const.tile([128, _NU, _SQ], F16, tag="qt")
        at_sb = const.tile([128, _NU, _SQ], F16, tag="at")
        ones_f32 = const.tile([1, 64], F32, tag="ones_f32")
        nc.vector.memset(ones_f32, 1.0)
        ones_row = const.tile([1, 64], F32R, tag="ones_row")
        nc.scalar.copy(ones_row, ones_f32)
        nc.vector.memset(vone, 1.0)  # ones cols 64/129 survive the V copies

        # wo tiles stay resident through attention so oproj is never DMA-gated
        wo_pool = ctx.enter_context(tc.tile_pool(name="wo_pool", bufs=16))
        wo_tiles = []

        # ---- phase 1: KV projection (own seq quarter) + gather ----------
        with (
            tc.tile_pool(name="xt_pool", bufs=1) as xt_pool,
            tc.tile_pool(name="wq_pool", bufs=4) as wq_pool,
        ):
            xt_sb = xt_pool.tile([128, _KC, _SQ], F16, tag="xt")
            nc.sync.dma_start(out=xt_sb, in_=xt_d[:, :, :])

            # prefetch the first Q-proj weight tiles ahead of everything
            # AG-gated so the sync DMA queue never stalls Q proj
            wq_tiles = {}
            for u in range(4):
                wqt = wq_pool.tile([128, _KC, 128], F16, tag="wqt")
                nc.sync.dma_start(out=wqt, in_=wq_d[u, :, :, :])
                wq_tiles[u] = wqt

            with (
                tc.tile_pool(name="wkv_pool", bufs=1) as wkv_pool,
                tc.tile_pool(name="kv_ps", bufs=4, space="PSUM") as kv_ps,
                tc.tile_pool(name="kvo_pool", bufs=4) as kvo_pool,
            ):
                wk_sb = wkv_pool.tile([128, _KC, 512], F16, tag="wk")
                wv_sb = wkv_pool.tile([128, _KC, 512], F16, tag="wv")
                nc.sync.dma_start(out=wk_sb, in_=wk_d[:, :, :])
                nc.sync.dma_start(out=wv_sb, in_=wv_d[:, :, :])
                for gg in range(4):
                    msl = slice(gg * 128, (gg + 1) * 128)
                    psk = kv_ps.tile([128, _SQ], F32, tag="psk")
                    psv = kv_ps.tile([128, _SQ], F32, tag="psv")
                    for kc in range(_KC):
                        st, sp = kc == 0, kc == _KC - 1
                        nc.tensor.matmul(
                            psk, wk_sb[:, kc, msl], xt_sb[:, kc, :], start=st, stop=sp
                        )
                        nc.tensor.matmul(
                            psv, wv_sb[:, kc, msl], xt_sb[:, kc, :], start=st, stop=sp
                        )
                    ko = kvo_pool.tile([128, _SQ], F16, tag="ko")
                    vo = kvo_pool.tile([128, _SQ], F16, tag="vo")
                    nc.scalar.copy(ko, psk)
                    nc.scalar.copy(vo, psv)
                    nc.sync.dma_start(out=kv_loc[gg * 128 : (gg + 1) * 128, :], in_=ko)
                    nc.sync.dma_start(
                        out=kv_loc[512 + gg * 128 : 512 + (gg + 1) * 128, :], in_=vo
                    )

            # gather K/V across the same-batch group (overlaps Q proj below)
            nc.gpsimd.collective_compute(
                "AllGather",
                mybir.AluOpType.bypass,
                replica_groups=[[0, 1, 2, 3], [4, 5, 6, 7]],
                ins=[kv_loc[:, :]],
                outs=[kv_full[:, :]],
            )

            # oproj weights: independent loads, queued before the AG-gated
            # unpack DMAs below
            for m in range(16):
                wot = wo_pool.tile([128, _KC, 128], F16, tag="wot")
                nc.sync.dma_start(out=wot, in_=wo_d[m, :, :, :])
                wo_tiles.append(wot)

            # ---- phase 2: Q projection (all 16 units) -------------------
            with tc.tile_pool(name="q_ps", bufs=3, space="PSUM") as q_ps:
                for u in range(_NU):
                    if u + 4 < _NU:
                        wqt = wq_pool.tile([128, _KC, 128], F16, tag="wqt")
                        nc.sync.dma_start(out=wqt, in_=wq_d[u + 4, :, :, :])
                        wq_tiles[u + 4] = wqt
                    psq = q_ps.tile([128, _SQ], F32, tag="psq")
                    for kc in range(_KC):
                        nc.tensor.matmul(
                            psq,
                            wq_tiles[u][:, kc, :],
                            xt_sb[:, kc, :],
                            start=kc == 0,
                            stop=kc == _KC - 1,
                        )
                    nc.scalar.copy(qt_sb[:, u, :], psq)

            # unpack K^T / V into gg-pair blocks, gg-major so attention can
            # start as soon as block 0 lands (these DMAs wait on the AG)
            with tc.tile_pool(name="vst_pool", bufs=4) as vst_pool:
                for gg in range(4):
                    for jj in range(4):
                        jsl = slice(jj * _SQ, (jj + 1) * _SQ)
                        nc.sync.dma_start(
                            out=kt_sb[:, gg, jsl],
                            in_=kv_full[
                                jj * 1024 + gg * 128 : jj * 1024 + (gg + 1) * 128, :
                            ],
                        )
                        # V natural via DMA-XBAR transpose (aligned staging,
                        # then copy around the fused ones columns)
                        r0 = jj * 1024 + 512 + gg * 128
                        for c in range(4):
                            sc = jj * 4 + c
                            csl = slice(c * 128, (c + 1) * 128)
                            vst = vst_pool.tile([128, 128], F16, tag="vst")
                            nc.sync.dma_start(
                                out=vst, in_=kv_full[r0 : r0 + 128, csl], transpose=True
                            )
                            nc.scalar.copy(vone[:, sc, gg, 0:64], vst[:, 0:64])
                            nc.scalar.copy(vone[:, sc, gg, 65:129], vst[:, 64:128])

        # ---- phase 3: attention (per head-pair unit, fully local) -------
        with (
            tc.tile_pool(name="p_pool", bufs=5) as p_pool,
            tc.tile_pool(name="ou_pool", bufs=4) as ou_pool,
            tc.tile_pool(name="nrm_pool", bufs=3) as nrm_pool,
            tc.tile_pool(name="atm_pool", bufs=2) as atm_pool,
            tc.tile_pool(name="sc_ps", bufs=2, space="PSUM") as sc_ps,
            tc.tile_pool(name="o_ps", bufs=2, space="PSUM") as o_ps,
        ):
            pend = []  # (u, ou_tile, rowsum_tile) awaiting normalization

            def finalize():
                u, ou_u, rs_u = pend.pop(0)
                rc = nrm_pool.tile([1, 1024], F32, tag="rc")
                nc.vector.reciprocal_approx_fast(rc, rs_u)
                rcr = nrm_pool.tile([1, 1024], F32R, tag="rcr")
                nc.scalar.copy(rcr, rc)
                rb_ps = sc_ps.tile([128, 1024], F32, tag="s_pair")
                nc.tensor.matmul(
                    rb_ps[0:64, 0:512],
                    ones_row,
                    rcr[0:1, 0:512],
                    start=True,
                    stop=True,
                )
                nc.tensor.matmul(
                    rb_ps[0:64, 512:1024],
                    ones_row,
                    rcr[0:1, 512:1024],
                    start=True,
                    stop=True,
                )
                rb_sb = nrm_pool.tile([64, 1024], F16, tag="rb")
                nc.vector.tensor_copy(rb_sb, rb_ps[0:64, :])
                atm = atm_pool.tile([64, 1024], F16, tag="atm")
                nc.vector.tensor_mul(atm, ou_u, rb_sb)
                # software DGE: keeps these writes off the (possibly busy)
                # sync DMA queue
                nc.gpsimd.dma_start(out=at_sb[0:64, u, :], in_=atm[:, 0:512])
                nc.gpsimd.dma_start(out=at_sb[64:128, u, :], in_=atm[:, 512:1024])

            # software pipeline: PV for (u, kt) trails its scores/exp by two
            # slots so the Act exp pipeline never gates the PE, including
            # across unit boundaries
            pvq = []  # pending (u, kt, pp) PV emissions
            o_pairs = {}

            def emit_pv():
                u, kt, pp = pvq.pop(0)
                gg = u // 4
                st, sp = kt == 0, kt == _ST - 1
                if st:
                    o_pairs[u] = o_ps.tile(
                        [65, 1024], F32, tag="o_pair", name=f"o_pair{u}"
                    )
                o_pair = o_pairs[u]
                nc.tensor.matmul(
                    o_pair[0:65, 0:512],
                    vone[:, kt, gg, 0:65],
                    pp[:, 0:512],
                    start=st,
                    stop=sp,
                )
                nc.tensor.matmul(
                    o_pair[0:65, 512:1024],
                    vone[:, kt, gg, 65:130],
                    pp[:, 512:1024],
                    start=st,
                    stop=sp,
                )
                if sp:
                    ou_u = ou_pool.tile([64, 1024], F16, tag="ou")
                    nc.vector.tensor_copy(ou_u, o_pair[0:64, :])
                    rs_u = nrm_pool.tile([1, 1024], F32, tag="rs")
                    nc.vector.tensor_copy(rs_u, o_pair[64:65, :])
                    del o_pairs[u]
                    pend.append((u, ou_u, rs_u))

            for u in range(_NU):
                gg = u // 4
                for kt in range(_ST):
                    ksl = slice(kt * 128, (kt + 1) * 128)
                    s_pair = sc_ps.tile([128, 1024], F32, tag="s_pair")
                    nc.tensor.matmul(
                        s_pair[:, 0:512],
                        kt_sb[0:64, gg, ksl],
                        qt_sb[0:64, u, :],
                        start=True,
                        stop=True,
                        tile_position=(0, 0),
                    )
                    nc.tensor.matmul(
                        s_pair[:, 512:1024],
                        kt_sb[64:128, gg, ksl],
                        qt_sb[64:128, u, :],
                        start=True,
                        stop=True,
                        tile_position=(64, 0),
                    )
                    pp = p_pool.tile([128, 1024], F16, tag="pp")
                    nc.scalar.activation(
                        pp, s_pair, mybir.ActivationFunctionType.Exp, scale=0.125
                    )
                    pvq.append((u, kt, pp))
                    if len(pvq) >= 3:
                        emit_pv()
                    if kt == 4 and pend:
                        finalize()
            while pvq:
                emit_pv()
            finalize()

        if dbg:
            nc.sync.dma_start(out=dbg_kt[:, :, :], in_=kt_sb)
            nc.sync.dma_start(out=dbg_vone[:, :, :, :], in_=vone)
            nc.sync.dma_start(out=dbg_qt[:, :, :], in_=qt_sb)
            nc.sync.dma_start(out=dbg_at[:, :, :], in_=at_sb)

        # ---- phase 4: output projection (own seq quarter, full output) --
        with (
            tc.tile_pool(name="wo_pool", bufs=3) as wo_pool,
            tc.tile_pool(name="y_pool", bufs=2) as y_pool,
            tc.tile_pool(name="y_ps", bufs=3, space="PSUM") as y_ps,
        ):
            for m in range(16):
                wot = wo_pool.tile([128, _KC, 128], F16, tag="wot")
                nc.sync.dma_start(out=wot, in_=wo_d[m, :, :, :])
                psy = y_ps.tile([128, _SQ], F32, tag="psy")
                for kc in range(_KC):
                    nc.tensor.matmul(
                        psy,
                        wot[:, kc, :],
                        at_sb[:, kc, :],
                        start=kc == 0,
                        stop=kc == _KC - 1,
                    )
                y_sb = y_pool.tile([128, _SQ], F32, tag="y")
                nc.scalar.copy(y_sb, psy)
                nc.sync.dma_start(out=yt_d[m * 128 : (m + 1) * 128, :], in_=y_sb)

    nc.compile()
    return nc


def kernel(X, Wq, Wk, Wv, Wo):
    global _nc_cache, _last_results
    X = np.asarray(X, dtype=np.float32)
    Wq = np.asarray(Wq, dtype=np.float32)
    Wk = np.asarray(Wk, dtype=np.float32)
    Wv = np.asarray(Wv, dtype=np.float32)
    Wo = np.asarray(Wo, dtype=np.float32)

    if _nc_cache is None:
        _nc_cache = _build()
    nc = _nc_cache

    # head-pair-unit permutation: unit u -> q heads (u, u+16)
    qcols = []
    for u in range(_NU):
        qcols.extend(range(u * _HD, (u + 1) * _HD))
        qcols.extend(range((u + 16) * _HD, (u + 17) * _HD))
    # kv-pair blocks: gg -> kv heads (gg, gg+4)
    kvcols = []
    for gg in range(4):
        kvcols.extend(range(gg * _HD, (gg + 1) * _HD))
        kvcols.extend(range((gg + 4) * _HD, (gg + 5) * _HD))

    wq_p = np.ascontiguousarray(
        Wq.astype(np.float16)[:, qcols]
        .reshape(_KC, 128, _NU, 128)
        .transpose(2, 1, 0, 3)
    )
    wk_p = np.ascontiguousarray(
        Wk.astype(np.float16)[:, kvcols].reshape(_KC, 128, 512).transpose(1, 0, 2)
    )
    wv_p = np.ascontiguousarray(
        Wv.astype(np.float16)[:, kvcols].reshape(_KC, 128, 512).transpose(1, 0, 2)
    )
    wo_p = np.ascontiguousarray(
        Wo.astype(np.float16)[qcols, :]
        .reshape(_KC, 128, 16, 128)
        .transpose(2, 1, 0, 3)
    )

    X16 = X.astype(np.float16)
    in_maps = []
    for c in range(_NCORES):
        b, j = divmod(c, 4)
        xt_q = np.ascontiguousarray(
            X16[b].T[:, j * _SQ : (j + 1) * _SQ]
            .reshape(_KC, 128, _SQ)
            .transpose(1, 0, 2)
        )
        in_maps.append(
            {"xt": xt_q, "wq": wq_p, "wk": wk_p, "wv": wv_p, "wo": wo_p}
        )

    trace = bool(os.environ.get("KERNEL_TRACE"))
    res = run_bass_kernel_spmd(
        nc, in_maps, core_ids=list(range(_NCORES)), trace=trace
    )
    _last_results = res

    Y = np.empty((_BATCH, _SEQ, _HIDDEN), dtype=np.float32)
    for c in range(_NCORES):
        b, j = divmod(c, 4)
        Y[b][j * _SQ : (j + 1) * _SQ, :] = res.results[c]["yt"].T
    return Y


# revision 21
# speedup vs baseline: 1.7916x; 1.1075x over previous
"""GQA attention kernel for 8 trn2 NeuronCores (Bass/Tile, SPMD) — V2.

Problem: X[2,2048,2048] fp32, Wq[2048,2048], Wk/Wv[2048,512], Wo[2048,2048].
  q/k/v proj -> GQA attention (32 Q heads, 8 KV heads, head_dim 64, no mask)
  -> out proj.

V2 sharding: SEQUENCE-parallel attention. Core c handles batch b=c//4 and
query-sequence quarter j=c%4 (positions [512j, 512j+512)) for ALL 32 heads.
  - KV proj: each core computes K^T,V^T for its seq quarter only, then one
    small fp16 AllGather (1MB in / 4MB out per core) within the same-batch
    4-core group produces the full K^T/V^T.  The gather overlaps Q proj.
  - Attention + output proj then run entirely core-locally: no late
    collective, no DRAM roundtrip for the attention output.
Head packing: unit u=0..15 pairs q-heads (u, u+16) whose KV heads are
(u//4, u//4+4) — packed on partition halves so score/PV matmuls use the
full 128-partition array (baseline's pairing trick).

All streamed matmul operands are fp16 (same PE rate as fp32r, half the
DMA/SBUF), accumulation stays fp32 in PSUM.  Softmax normalization is kept
entirely off the PE critical path: rowsums come from a fused ones-column in
the PV lhsT; reciprocals use the fast custom-DVE op; the per-column
broadcast runs as a tiny fp32r matmul emitted one unit late so its DVE
dependency is long resolved by the time the PE reaches it.
"""

import os
import sys
import types
from contextlib import ExitStack

import numpy as np

_HIDDEN = 2048
_SEQ = 2048
_BATCH = 2
_NH = 32  # query heads
_NKV = 8
_HD = 64  # head dim
_NCORES = 8

_SQ = 512  # seq quarter per core
_KC = _HIDDEN // 128  # 16 contraction chunks
_NU = 16  # head-pair units per core (all 32 heads)
_ST = _SEQ // 128  # 16 key chunks of 128


def _install_ntff_hook():
    """antenv in this image lacks axon_hooks; synthesize it so the axon
    NTFF profiling path works when tracing is requested."""
    try:
        import antenv

        try:
            from antenv import axon_hooks  # noqa: F401

            return
        except ImportError:
            pass
        mod = types.ModuleType("antenv.axon_hooks")
        mod._hook = None
        mod.set_axon_ntff_profile_hook = lambda h: setattr(mod, "_hook", h)
        mod.get_axon_ntff_profile_hook = lambda: mod._hook
        sys.modules["antenv.axon_hooks"] = mod
        antenv.axon_hooks = mod
        from trn_agent_boot.trn_boot import _ntff_profile_via_ctypes

        so = "/opt/axon/libaxon_pjrt.so"
        if os.path.exists(so):
            mod.set_axon_ntff_profile_hook(_ntff_profile_via_ctypes(so))
    except Exception:
        pass


_install_ntff_hook()

import concourse.bass as bass  # noqa: E402
import concourse.tile as tile  # noqa: E402
from concourse import bacc, bass_utils, mybir  # noqa: E402
from concourse.bass_utils import run_bass_kernel_spmd  # noqa: E402

# no S3 bucket in this container; keep trace artifacts local
bass_utils.upload_artifacts = lambda tmpdir: tmpdir

F32 = mybir.dt.float32
F32R = mybir.dt.float32r
F16 = mybir.dt.float16

_nc_cache = None
_last_results = None
_USE_PBCAST = os.environ.get("KERNEL_NO_PBCAST") is None


def _build():
    nc = bacc.Bacc("TRN2", target_bir_lowering=False, debug=False, num_devices=8)

    xt_d = nc.declare_dram_parameter("xt", [128, _KC, _SQ], F16, isOutput=False)
    wk_d = nc.declare_dram_parameter("wk", [128, _KC, 512], F16, isOutput=False)
    wv_d = nc.declare_dram_parameter("wv", [128, _KC, 512], F16, isOutput=False)
    wq_d = nc.declare_dram_parameter("wq", [_NU, 128, _KC, 128], F16, isOutput=False)
    wo_d = nc.declare_dram_parameter("wo", [16, 128, _KC, 128], F16, isOutput=False)
    yt_d = nc.declare_dram_parameter("yt", [_HIDDEN, _SQ], F32, isOutput=True)

    # K^T local [512, SQ] then V^T local [512, SQ]
    kv_loc = nc.dram_tensor("kv_loc", [1024, _SQ], F16)
    kv_full = nc.dram_tensor("kv_full", [4096, _SQ], F16)

    dbg = bool(os.environ.get("KERNEL_DEBUG"))
    if dbg:
        dbg_kt = nc.declare_dram_parameter("dbg_kt", [128, 4, _SEQ], F16, isOutput=True)
        dbg_vone = nc.declare_dram_parameter(
            "dbg_vone", [128, _ST, 4, 130], F16, isOutput=True
        )
        dbg_qt = nc.declare_dram_parameter(
            "dbg_qt", [128, _NU, _SQ], F16, isOutput=True
        )
        dbg_at = nc.declare_dram_parameter(
            "dbg_at", [128, _NU, _SQ], F16, isOutput=True
        )

    with (
        tile.TileContext(nc) as tc,
        ExitStack() as ctx,
        nc.allow_low_precision(reason="fp16 streams; fp32 PSUM accumulation"),
    ):
        const = ctx.enter_context(tc.tile_pool(name="const", bufs=1))

        # ---- persistent SBUF tensors -------------------------------------
        kt_sb = const.tile([128, 4, _SEQ], F16, tag="kt")  # gg-pair blocks
        vone = const.tile([128, _ST, 4, 130], F16, tag="vone")
        qt_sb = const.tile([128, _NU, _SQ], F16, tag="qt")
        at_sb = const.tile([128, _NU, _SQ], F16, tag="at")
        ones_f32 = const.tile([1, 64], F32, tag="ones_f32")
        nc.vector.memset(ones_f32, 1.0)
        ones_row = const.tile([1, 64], F32R, tag="ones_row")
        nc.scalar.copy(ones_row, ones_f32)
        nc.vector.memset(vone, 1.0)  # ones cols 64/129 survive the V copies

        # wo tiles stay resident through attention so oproj is never DMA-gated
        wo_pool = ctx.enter_context(tc.tile_pool(name="wo_pool", bufs=16))
        wo_tiles = []

        # ---- phase 1: KV projection (own seq quarter) + gather ----------
        with (
            tc.tile_pool(name="xt_pool", bufs=1) as xt_pool,
            tc.tile_pool(name="wq_pool", bufs=4) as wq_pool,
        ):
            xt_sb = xt_pool.tile([128, _KC, _SQ], F16, tag="xt")
            nc.sync.dma_start(out=xt_sb, in_=xt_d[:, :, :])

            # prefetch the first Q-proj weight tiles ahead of everything
            # AG-gated so the sync DMA queue never stalls Q proj
            wq_tiles = {}
            for u in range(4):
                wqt = wq_pool.tile([128, _KC, 128], F16, tag="wqt")
                nc.sync.dma_start(out=wqt, in_=wq_d[u, :, :, :])
                wq_tiles[u] = wqt

            with (
                tc.tile_pool(name="wkv_pool", bufs=1) as wkv_pool,
                tc.tile_pool(name="kv_ps", bufs=4, space="PSUM") as kv_ps,
                tc.tile_pool(name="kvo_pool", bufs=4) as kvo_pool,
            ):
                wk_sb = wkv_pool.tile([128, _KC, 512], F16, tag="wk")
                wv_sb = wkv_pool.tile([128, _KC, 512], F16, tag="wv")
                nc.sync.dma_start(out=wk_sb, in_=wk_d[:, :, :])
                nc.sync.dma_start(out=wv_sb, in_=wv_d[:, :, :])
                for gg in range(4):
                    msl = slice(gg * 128, (gg + 1) * 128)
                    psk = kv_ps.tile([128, _SQ], F32, tag="psk")
                    psv = kv_ps.tile([128, _SQ], F32, tag="psv")
                    for kc in range(_KC):
                        st, sp = kc == 0, kc == _KC - 1
                        nc.tensor.matmul(
                            psk, wk_sb[:, kc, msl], xt_sb[:, kc, :], start=st, stop=sp
                        )
                        nc.tensor.matmul(
                            psv, wv_sb[:, kc, msl], xt_sb[:, kc, :], start=st, stop=sp
                        )
                    ko = kvo_pool.tile([128, _SQ], F16, tag="ko")
                    vo = kvo_pool.tile([128, _SQ], F16, tag="vo")
                    nc.scalar.copy(ko, psk)
                    nc.scalar.copy(vo, psv)
                    # Act hwdge queue: keeps the sync queue free-flowing for
                    # the weight loads (these writes wait on local compute)
                    nc.scalar.dma_start(
                        out=kv_loc[gg * 128 : (gg + 1) * 128, :], in_=ko
                    )
                    nc.scalar.dma_start(
                        out=kv_loc[512 + gg * 128 : 512 + (gg + 1) * 128, :], in_=vo
                    )

            # gather K/V across the same-batch group (overlaps Q proj below)
            nc.gpsimd.collective_compute(
                "AllGather",
                mybir.AluOpType.bypass,
                replica_groups=[[0, 1, 2, 3], [4, 5, 6, 7]],
                ins=[kv_loc[:, :]],
                outs=[kv_full[:, :]],
            )

            # remaining Q weights first (needed soonest), then oproj weights
            for u in range(4, _NU):
                wqt = wq_pool.tile([128, _KC, 128], F16, tag="wqt", name=f"wq{u}")
                nc.sync.dma_start(out=wqt, in_=wq_d[u, :, :, :])
                wq_tiles[u] = wqt
            for m in range(16):
                wot = wo_pool.tile([128, _KC, 128], F16, tag="wot")
                nc.sync.dma_start(out=wot, in_=wo_d[m, :, :, :])
                wo_tiles.append(wot)

            # ---- phase 2: Q projection (all 16 units) -------------------
            with tc.tile_pool(name="q_ps", bufs=3, space="PSUM") as q_ps:
                for u in range(_NU):
                    psq = q_ps.tile([128, _SQ], F32, tag="psq")
                    for kc in range(_KC):
                        nc.tensor.matmul(
                            psq,
                            wq_tiles[u][:, kc, :],
                            xt_sb[:, kc, :],
                            start=kc == 0,
                            stop=kc == _KC - 1,
                        )
                    nc.scalar.copy(qt_sb[:, u, :], psq)

            # unpack K^T / V into gg-pair blocks, gg-major so attention can
            # start as soon as block 0 lands (these DMAs wait on the AG)
            with tc.tile_pool(name="vst_pool", bufs=4) as vst_pool:
                for gg in range(4):
                    for jj in range(4):
                        jsl = slice(jj * _SQ, (jj + 1) * _SQ)
                        nc.sync.dma_start(
                            out=kt_sb[:, gg, jsl],
                            in_=kv_full[
                                jj * 1024 + gg * 128 : jj * 1024 + (gg + 1) * 128, :
                            ],
                        )
                        # V natural via DMA-XBAR transpose (aligned staging,
                        # then copy around the fused ones columns)
                        r0 = jj * 1024 + 512 + gg * 128
                        for c in range(4):
                            sc = jj * 4 + c
                            csl = slice(c * 128, (c + 1) * 128)
                            vst = vst_pool.tile([128, 128], F16, tag="vst")
                            nc.sync.dma_start(
                                out=vst, in_=kv_full[r0 : r0 + 128, csl], transpose=True
                            )
                            nc.scalar.copy(vone[:, sc, gg, 0:64], vst[:, 0:64])
                            nc.scalar.copy(vone[:, sc, gg, 65:129], vst[:, 64:128])

        # ---- phase 3: attention (per head-pair unit, fully local) -------
        with (
            tc.tile_pool(name="p_pool", bufs=5) as p_pool,
            tc.tile_pool(name="ou_pool", bufs=4) as ou_pool,
            tc.tile_pool(name="nrm_pool", bufs=3) as nrm_pool,
            tc.tile_pool(name="atm_pool", bufs=2) as atm_pool,
            tc.tile_pool(name="sc_ps", bufs=2, space="PSUM") as sc_ps,
            tc.tile_pool(name="o_ps", bufs=2, space="PSUM") as o_ps,
        ):
            pend = []  # (u, ou_tile, rowsum_tile) awaiting normalization

            def finalize():
                u, ou_u, rs_u = pend.pop(0)
                rc = nrm_pool.tile([1, 1024], F32, tag="rc")
                nc.vector.reciprocal_approx_fast(rc, rs_u)
                if _USE_PBCAST:
                    rb_g = nrm_pool.tile([64, 1024], F32, tag="rbg")
                    nc.gpsimd.partition_broadcast(rb_g, rc)
                    rb_in = rb_g
                else:
                    rcr = nrm_pool.tile([1, 1024], F32R, tag="rcr")
                    nc.scalar.copy(rcr, rc)
                    rb_ps = sc_ps.tile([128, 1024], F32, tag="s_pair")
                    nc.tensor.matmul(
                        rb_ps[0:64, 0:512],
                        ones_row,
                        rcr[0:1, 0:512],
                        start=True,
                        stop=True,
                    )
                    nc.tensor.matmul(
                        rb_ps[0:64, 512:1024],
                        ones_row,
                        rcr[0:1, 512:1024],
                        start=True,
                        stop=True,
                    )
                    rb_sb = nrm_pool.tile([64, 1024], F16, tag="rb")
                    nc.vector.tensor_copy(rb_sb, rb_ps[0:64, :])
                    rb_in = rb_sb
                atm = atm_pool.tile([64, 1024], F16, tag="atm")
                nc.vector.tensor_mul(atm, ou_u, rb_in)
                # software DGE: keeps these writes off the (possibly busy)
                # sync DMA queue
                nc.gpsimd.dma_start(out=at_sb[0:64, u, :], in_=atm[:, 0:512])
                nc.gpsimd.dma_start(out=at_sb[64:128, u, :], in_=atm[:, 512:1024])

            # software pipeline: PV for (u, kt) trails its scores/exp by two
            # slots so the Act exp pipeline never gates the PE, including
            # across unit boundaries
            pvq = []  # pending (u, kt, pp) PV emissions
            o_pairs = {}

            def emit_pv():
                u, kt, pp = pvq.pop(0)
                gg = u // 4
                st, sp = kt == 0, kt == _ST - 1
                if st:
                    o_pairs[u] = o_ps.tile(
                        [65, 1024], F32, tag="o_pair", name=f"o_pair{u}"
                    )
                o_pair = o_pairs[u]
                nc.tensor.matmul(
                    o_pair[0:65, 0:512],
                    vone[:, kt, gg, 0:65],
                    pp[:, 0:512],
                    start=st,
                    stop=sp,
                )
                nc.tensor.matmul(
                    o_pair[0:65, 512:1024],
                    vone[:, kt, gg, 65:130],
                    pp[:, 512:1024],
                    start=st,
                    stop=sp,
                )
                if sp:
                    ou_u = ou_pool.tile([64, 1024], F16, tag="ou")
                    nc.vector.tensor_copy(ou_u, o_pair[0:64, :])
                    rs_u = nrm_pool.tile([1, 1024], F32, tag="rs")
                    nc.vector.tensor_copy(rs_u, o_pair[64:65, :])
                    del o_pairs[u]
                    pend.append((u, ou_u, rs_u))

            for u in range(_NU):
                gg = u // 4
                for kt in range(_ST):
                    ksl = slice(kt * 128, (kt + 1) * 128)
                    s_pair = sc_ps.tile([128, 1024], F32, tag="s_pair")
                    nc.tensor.matmul(
                        s_pair[:, 0:512],
                        kt_sb[0:64, gg, ksl],
                        qt_sb[0:64, u, :],
                        start=True,
                        stop=True,
                        tile_position=(0, 0),
                    )
                    nc.tensor.matmul(
                        s_pair[:, 512:1024],
                        kt_sb[64:128, gg, ksl],
                        qt_sb[64:128, u, :],
                        start=True,
                        stop=True,
                        tile_position=(64, 0),
                    )
                    pp = p_pool.tile([128, 1024], F16, tag="pp")
                    nc.scalar.activation(
                        pp, s_pair, mybir.ActivationFunctionType.Exp, scale=0.125
                    )
                    pvq.append((u, kt, pp))
                    if len(pvq) >= 3:
                        emit_pv()
                    if kt == 4 and pend:
                        finalize()
            while pvq:
                emit_pv()
            finalize()

        if dbg:
            nc.sync.dma_start(out=dbg_kt[:, :, :], in_=kt_sb)
            nc.sync.dma_start(out=dbg_vone[:, :, :, :], in_=vone)
            nc.sync.dma_start(out=dbg_qt[:, :, :], in_=qt_sb)
            nc.sync.dma_start(out=dbg_at[:, :, :], in_=at_sb)

        # ---- phase 4: output projection (own seq quarter, full output) --
        with (
            tc.tile_pool(name="wo_pool", bufs=3) as wo_pool,
            tc.tile_pool(name="y_pool", bufs=2) as y_pool,
            tc.tile_pool(name="y_ps", bufs=3, space="PSUM") as y_ps,
        ):
            for m in range(16):
                wot = wo_pool.tile([128, _KC, 128], F16, tag="wot")
                nc.sync.dma_start(out=wot, in_=wo_d[m, :, :, :])
                psy = y_ps.tile([128, _SQ], F32, tag="psy")
                for kc in range(_KC):
                    nc.tensor.matmul(
                        psy,
                        wot[:, kc, :],
                        at_sb[:, kc, :],
                        start=kc == 0,
                        stop=kc == _KC - 1,
                    )
                y_sb = y_pool.tile([128, _SQ], F32, tag="y")
                nc.scalar.copy(y_sb, psy)
                nc.sync.dma_start(out=yt_d[m * 128 : (m + 1) * 128, :], in_=y_sb)

    nc.compile()
    return nc


def kernel(X, Wq, Wk, Wv, Wo):
    global _nc_cache, _last_results
    X = np.asarray(X, dtype=np.float32)
    Wq = np.asarray(Wq, dtype=np.float32)
    Wk = np.asarray(Wk, dtype=np.float32)
    Wv = np.asarray(Wv, dtype=np.float32)
    Wo = np.asarray(Wo, dtype=np.float32)

    if _nc_cache is None:
        _nc_cache = _build()
    nc = _nc_cache

    # head-pair-unit permutation: unit u -> q heads (u, u+16)
    qcols = []
    for u in range(_NU):
        qcols.extend(range(u * _HD, (u + 1) * _HD))
        qcols.extend(range((u + 16) * _HD, (u + 17) * _HD))
    # kv-pair blocks: gg -> kv heads (gg, gg+4)
    kvcols = []
    for gg in range(4):
        kvcols.extend(range(gg * _HD, (gg + 1) * _HD))
        kvcols.extend(range((gg + 4) * _HD, (gg + 5) * _HD))

    wq_p = np.ascontiguousarray(
        Wq.astype(np.float16)[:, qcols]
        .reshape(_KC, 128, _NU, 128)
        .transpose(2, 1, 0, 3)
    )
    wk_p = np.ascontiguousarray(
        Wk.astype(np.float16)[:, kvcols].reshape(_KC, 128, 512).transpose(1, 0, 2)
    )
    wv_p = np.ascontiguousarray(
        Wv.astype(np.float16)[:, kvcols].reshape(_KC, 128, 512).transpose(1, 0, 2)
    )
    wo_p = np.ascontiguousarray(
        Wo.astype(np.float16)[qcols, :]
        .reshape(_KC, 128, 16, 128)
        .transpose(2, 1, 0, 3)
    )

    X16 = X.astype(np.float16)
    in_maps = []
    for c in range(_NCORES):
        b, j = divmod(c, 4)
        xt_q = np.ascontiguousarray(
            X16[b].T[:, j * _SQ : (j + 1) * _SQ]
            .reshape(_KC, 128, _SQ)
            .transpose(1, 0, 2)
        )
        in_maps.append(
            {"xt": xt_q, "wq": wq_p, "wk": wk_p, "wv": wv_p, "wo": wo_p}
        )

    trace = bool(os.environ.get("KERNEL_TRACE"))
    res = run_bass_kernel_spmd(
        nc, in_maps, core_ids=list(range(_NCORES)), trace=trace
    )
    _last_results = res

    Y = np.empty((_BATCH, _SEQ, _HIDDEN), dtype=np.float32)
    for c in range(_NCORES):
        b, j = divmod(c, 4)
        Y[b][j * _SQ : (j + 1) * _SQ, :] = res.results[c]["yt"].T
    return Y


# revision 23
# speedup vs baseline: 2.1749x; 1.2139x over previous
"""GQA attention kernel for 8 trn2 NeuronCores (Bass/Tile, SPMD) — V2.

Problem: X[2,2048,2048] fp32, Wq[2048,2048], Wk/Wv[2048,512], Wo[2048,2048].
  q/k/v proj -> GQA attention (32 Q heads, 8 KV heads, head_dim 64, no mask)
  -> out proj.

V2 sharding: SEQUENCE-parallel attention. Core c handles batch b=c//4 and
query-sequence quarter j=c%4 (positions [512j, 512j+512)) for ALL 32 heads.
  - KV proj: each core computes K^T,V^T for its seq quarter only, then one
    small fp16 AllGather (1MB in / 4MB out per core) within the same-batch
    4-core group produces the full K^T/V^T.  The gather overlaps Q proj.
  - Attention + output proj then run entirely core-locally: no late
    collective, no DRAM roundtrip for the attention output.
Head packing: unit u=0..15 pairs q-heads (u, u+16) whose KV heads are
(u//4, u//4+4) — packed on partition halves so score/PV matmuls use the
full 128-partition array (baseline's pairing trick).

All streamed matmul operands are fp16 (same PE rate as fp32r, half the
DMA/SBUF), accumulation stays fp32 in PSUM.  Softmax normalization is kept
entirely off the PE critical path: rowsums come from a fused ones-column in
the PV lhsT; reciprocals use the fast custom-DVE op; the per-column
broadcast runs as a tiny fp32r matmul emitted one unit late so its DVE
dependency is long resolved by the time the PE reaches it.
"""

import os
import sys
import types
from contextlib import ExitStack

import numpy as np

_HIDDEN = 2048
_SEQ = 2048
_BATCH = 2
_NH = 32  # query heads
_NKV = 8
_HD = 64  # head dim
_NCORES = 8

_SQ = 512  # seq quarter per core
_KC = _HIDDEN // 128  # 16 contraction chunks
_NU = 16  # head-pair units per core (all 32 heads)
_ST = _SEQ // 128  # 16 key chunks of 128


def _install_ntff_hook():
    """antenv in this image lacks axon_hooks; synthesize it so the axon
    NTFF profiling path works when tracing is requested."""
    try:
        import antenv

        try:
            from antenv import axon_hooks  # noqa: F401

            return
        except ImportError:
            pass
        mod = types.ModuleType("antenv.axon_hooks")
        mod._hook = None
        mod.set_axon_ntff_profile_hook = lambda h: setattr(mod, "_hook", h)
        mod.get_axon_ntff_profile_hook = lambda: mod._hook
        sys.modules["antenv.axon_hooks"] = mod
        antenv.axon_hooks = mod
        from trn_agent_boot.trn_boot import _ntff_profile_via_ctypes

        so = "/opt/axon/libaxon_pjrt.so"
        if os.path.exists(so):
            mod.set_axon_ntff_profile_hook(_ntff_profile_via_ctypes(so))
    except Exception:
        pass


_install_ntff_hook()

import concourse.bass as bass  # noqa: E402
import concourse.tile as tile  # noqa: E402
from concourse import bacc, bass_utils, mybir  # noqa: E402
from concourse.bass_utils import run_bass_kernel_spmd  # noqa: E402

# no S3 bucket in this container; keep trace artifacts local
bass_utils.upload_artifacts = lambda tmpdir: tmpdir

F32 = mybir.dt.float32
F32R = mybir.dt.float32r
F16 = mybir.dt.float16

_nc_cache = None
_last_results = None
_USE_PBCAST = os.environ.get("KERNEL_NO_PBCAST") is None


def _build():
    nc = bacc.Bacc("TRN2", target_bir_lowering=False, debug=False, num_devices=8)

    xt_d = nc.declare_dram_parameter("xt", [128, _KC, _SQ], F16, isOutput=False)
    wk_d = nc.declare_dram_parameter("wk", [128, _KC, 512], F16, isOutput=False)
    wv_d = nc.declare_dram_parameter("wv", [128, _KC, 512], F16, isOutput=False)
    wq_d = nc.declare_dram_parameter("wq", [_NU, 128, _KC, 128], F16, isOutput=False)
    wo_d = nc.declare_dram_parameter("wo", [16, 128, _KC, 128], F16, isOutput=False)
    yt_d = nc.declare_dram_parameter("yt", [_HIDDEN, _SQ], F32, isOutput=True)

    # K^T local [512, SQ] then V^T local [512, SQ]
    kv_loc = nc.dram_tensor("kv_loc", [1024, _SQ], F16)
    kv_full = nc.dram_tensor("kv_full", [4096, _SQ], F16)

    dbg = bool(os.environ.get("KERNEL_DEBUG"))
    if dbg:
        dbg_kt = nc.declare_dram_parameter("dbg_kt", [128, 4, _SEQ], F16, isOutput=True)
        dbg_vone = nc.declare_dram_parameter(
            "dbg_vone", [128, _ST, 4, 130], F16, isOutput=True
        )
        dbg_qt = nc.declare_dram_parameter(
            "dbg_qt", [128, _NU, _SQ], F16, isOutput=True
        )
        dbg_at = nc.declare_dram_parameter(
            "dbg_at", [128, _NU, _SQ], F16, isOutput=True
        )

    with (
        tile.TileContext(nc) as tc,
        ExitStack() as ctx,
        nc.allow_low_precision(reason="fp16 streams; fp32 PSUM accumulation"),
    ):
        const = ctx.enter_context(tc.tile_pool(name="const", bufs=1))

        # ---- persistent SBUF tensors -------------------------------------
        kt_sb = const.tile([128, 4, _SEQ], F16, tag="kt")  # gg-pair blocks
        vone = const.tile([128, _ST, 4, 130], F16, tag="vone")
        qt_sb = const.tile([128, _NU, _SQ], F16, tag="qt")
        at_sb = const.tile([128, _NU, _SQ], F16, tag="at")
        ones_f32 = const.tile([1, 64], F32, tag="ones_f32")
        nc.vector.memset(ones_f32, 1.0)
        ones_row = const.tile([1, 64], F32R, tag="ones_row")
        nc.scalar.copy(ones_row, ones_f32)
        nc.vector.memset(vone, 1.0)  # ones cols 64/129 survive the V copies

        # wo tiles stay resident through attention so oproj is never DMA-gated
        wo_pool = ctx.enter_context(tc.tile_pool(name="wo_pool", bufs=16))
        wo_tiles = []

        # ---- phase 1: KV projection (own seq quarter) + gather ----------
        with (
            tc.tile_pool(name="xt_pool", bufs=1) as xt_pool,
            tc.tile_pool(name="wq_pool", bufs=4) as wq_pool,
        ):
            xt_sb = xt_pool.tile([128, _KC, _SQ], F16, tag="xt")
            nc.sync.dma_start(out=xt_sb, in_=xt_d[:, :, :])

            wq_tiles = {}
            with (
                tc.tile_pool(name="wkv_pool", bufs=1) as wkv_pool,
                tc.tile_pool(name="kv_ps", bufs=4, space="PSUM") as kv_ps,
                tc.tile_pool(name="kvo_pool", bufs=4) as kvo_pool,
            ):
                wk_sb = wkv_pool.tile([128, _KC, 512], F16, tag="wk")
                wv_sb = wkv_pool.tile([128, _KC, 512], F16, tag="wv")
                nc.sync.dma_start(out=wk_sb, in_=wk_d[:, :, :])
                nc.sync.dma_start(out=wv_sb, in_=wv_d[:, :, :])
                # prefetch the first Q-proj weight tiles behind wk/wv so the
                # sync DMA queue never stalls Q proj
                for u in range(4):
                    wqt = wq_pool.tile([128, _KC, 128], F16, tag="wqt")
                    nc.sync.dma_start(out=wqt, in_=wq_d[u, :, :, :])
                    wq_tiles[u] = wqt
                # K^T: kv-dims on partitions (pair-blocks), seq on free axis
                for gg in range(4):
                    msl = slice(gg * 128, (gg + 1) * 128)
                    psk = kv_ps.tile([128, _SQ], F32, tag="psk")
                    for kc in range(_KC):
                        nc.tensor.matmul(
                            psk,
                            wk_sb[:, kc, msl],
                            xt_sb[:, kc, :],
                            start=kc == 0,
                            stop=kc == _KC - 1,
                        )
                    ko = kvo_pool.tile([128, _SQ], F16, tag="ko")
                    nc.scalar.copy(ko, psk)
                    # Act hwdge queue: keeps the sync queue free-flowing for
                    # the weight loads (these writes wait on local compute)
                    nc.scalar.dma_start(
                        out=kv_loc[gg * 128 : (gg + 1) * 128, :], in_=ko
                    )
                # V natural (seq on partitions): lhsT = X^T chunk, rhs = Wv —
                # no transposes needed anywhere downstream
                for s in range(4):
                    ssl = slice(s * 128, (s + 1) * 128)
                    psv = kv_ps.tile([128, _SQ], F32, tag="psv")
                    for kc in range(_KC):
                        nc.tensor.matmul(
                            psv,
                            xt_sb[:, kc, ssl],
                            wv_sb[:, kc, :],
                            start=kc == 0,
                            stop=kc == _KC - 1,
                        )
                    vo = kvo_pool.tile([128, _SQ], F16, tag="vo")
                    nc.scalar.copy(vo, psv)
                    nc.scalar.dma_start(
                        out=kv_loc[512 + s * 128 : 512 + (s + 1) * 128, :], in_=vo
                    )

            # gather K/V across the same-batch group (overlaps Q proj below)
            nc.gpsimd.collective_compute(
                "AllGather",
                mybir.AluOpType.bypass,
                replica_groups=[[0, 1, 2, 3], [4, 5, 6, 7]],
                ins=[kv_loc[:, :]],
                outs=[kv_full[:, :]],
            )

            # remaining Q weights first (needed soonest), then oproj weights
            for u in range(4, _NU):
                wqt = wq_pool.tile([128, _KC, 128], F16, tag="wqt", name=f"wq{u}")
                nc.sync.dma_start(out=wqt, in_=wq_d[u, :, :, :])
                wq_tiles[u] = wqt
            for m in range(16):
                wot = wo_pool.tile([128, _KC, 128], F16, tag="wot")
                nc.sync.dma_start(out=wot, in_=wo_d[m, :, :, :])
                wo_tiles.append(wot)

            # ---- phase 2: Q projection (all 16 units) -------------------
            with tc.tile_pool(name="q_ps", bufs=3, space="PSUM") as q_ps:
                for u in range(_NU):
                    psq = q_ps.tile([128, _SQ], F32, tag="psq")
                    for kc in range(_KC):
                        nc.tensor.matmul(
                            psq,
                            wq_tiles[u][:, kc, :],
                            xt_sb[:, kc, :],
                            start=kc == 0,
                            stop=kc == _KC - 1,
                        )
                    nc.scalar.copy(qt_sb[:, u, :], psq)

            # unpack K^T / V into gg-pair blocks, gg-major so attention can
            # start as soon as block 0 lands (these DMAs wait on the AG)
            for gg in range(4):
                for jj in range(4):
                    jsl = slice(jj * _SQ, (jj + 1) * _SQ)
                    nc.sync.dma_start(
                        out=kt_sb[:, gg, jsl],
                        in_=kv_full[
                            jj * 1024 + gg * 128 : jj * 1024 + (gg + 1) * 128, :
                        ],
                    )
                    for h in range(2):
                        c0 = 128 * gg + 64 * h
                        nc.sync.dma_start(
                            out=vone[:, 4 * jj : 4 * jj + 4, gg, 65 * h : 65 * h + 64],
                            in_=kv_full[
                                jj * 1024 + 512 : jj * 1024 + 1024, c0 : c0 + 64
                            ].rearrange("(sc p) d -> p sc d", p=128),
                        )

        # ---- phase 3: attention (per head-pair unit, fully local) -------
        with (
            tc.tile_pool(name="p_pool", bufs=5) as p_pool,
            tc.tile_pool(name="ou_pool", bufs=4) as ou_pool,
            tc.tile_pool(name="nrm_pool", bufs=3) as nrm_pool,
            tc.tile_pool(name="atm_pool", bufs=2) as atm_pool,
            tc.tile_pool(name="sc_ps", bufs=2, space="PSUM") as sc_ps,
            tc.tile_pool(name="o_ps", bufs=2, space="PSUM") as o_ps,
        ):
            pend = []  # (u, ou_tile, rowsum_tile) awaiting normalization

            def finalize():
                u, ou_u, rs_u = pend.pop(0)
                rc = nrm_pool.tile([1, 1024], F32, tag="rc")
                nc.vector.reciprocal_approx_fast(rc, rs_u)
                if _USE_PBCAST:
                    rb_g = nrm_pool.tile([64, 1024], F32, tag="rbg")
                    nc.gpsimd.partition_broadcast(rb_g, rc)
                    rb_in = rb_g
                else:
                    rcr = nrm_pool.tile([1, 1024], F32R, tag="rcr")
                    nc.scalar.copy(rcr, rc)
                    rb_ps = sc_ps.tile([128, 1024], F32, tag="s_pair")
                    nc.tensor.matmul(
                        rb_ps[0:64, 0:512],
                        ones_row,
                        rcr[0:1, 0:512],
                        start=True,
                        stop=True,
                    )
                    nc.tensor.matmul(
                        rb_ps[0:64, 512:1024],
                        ones_row,
                        rcr[0:1, 512:1024],
                        start=True,
                        stop=True,
                    )
                    rb_sb = nrm_pool.tile([64, 1024], F16, tag="rb")
                    nc.vector.tensor_copy(rb_sb, rb_ps[0:64, :])
                    rb_in = rb_sb
                atm = atm_pool.tile([64, 1024], F16, tag="atm")
                nc.vector.tensor_mul(atm, ou_u, rb_in)
                # software DGE: keeps these writes off the (possibly busy)
                # sync DMA queue
                nc.gpsimd.dma_start(out=at_sb[0:64, u, :], in_=atm[:, 0:512])
                nc.gpsimd.dma_start(out=at_sb[64:128, u, :], in_=atm[:, 512:1024])

            # software pipeline: PV for (u, kt) trails its scores/exp by two
            # slots so the Act exp pipeline never gates the PE, including
            # across unit boundaries
            pvq = []  # pending (u, kt, pp) PV emissions
            o_pairs = {}

            def emit_pv():
                u, kt, pp = pvq.pop(0)
                gg = u // 4
                st, sp = kt == 0, kt == _ST - 1
                if st:
                    o_pairs[u] = o_ps.tile(
                        [65, 1024], F32, tag="o_pair", name=f"o_pair{u}"
                    )
                o_pair = o_pairs[u]
                nc.tensor.matmul(
                    o_pair[0:65, 0:512],
                    vone[:, kt, gg, 0:65],
                    pp[:, 0:512],
                    start=st,
                    stop=sp,
                )
                nc.tensor.matmul(
                    o_pair[0:65, 512:1024],
                    vone[:, kt, gg, 65:130],
                    pp[:, 512:1024],
                    start=st,
                    stop=sp,
                )
                if sp:
                    ou_u = ou_pool.tile([64, 1024], F16, tag="ou")
                    nc.vector.tensor_copy(ou_u, o_pair[0:64, :])
                    rs_u = nrm_pool.tile([1, 1024], F32, tag="rs")
                    nc.vector.tensor_copy(rs_u, o_pair[64:65, :])
                    del o_pairs[u]
                    pend.append((u, ou_u, rs_u))

            for u in range(_NU):
                gg = u // 4
                for kt in range(_ST):
                    ksl = slice(kt * 128, (kt + 1) * 128)
                    s_pair = sc_ps.tile([128, 1024], F32, tag="s_pair")
                    nc.tensor.matmul(
                        s_pair[:, 0:512],
                        kt_sb[0:64, gg, ksl],
                        qt_sb[0:64, u, :],
                        start=True,
                        stop=True,
                        tile_position=(0, 0),
                    )
                    nc.tensor.matmul(
                        s_pair[:, 512:1024],
                        kt_sb[64:128, gg, ksl],
                        qt_sb[64:128, u, :],
                        start=True,
                        stop=True,
                        tile_position=(64, 0),
                    )
                    pp = p_pool.tile([128, 1024], F16, tag="pp")
                    nc.scalar.activation(
                        pp, s_pair, mybir.ActivationFunctionType.Exp, scale=0.125
                    )
                    pvq.append((u, kt, pp))
                    if len(pvq) >= 3:
                        emit_pv()
                    if kt == 4 and pend:
                        finalize()
            while pvq:
                emit_pv()
            finalize()

        if dbg:
            nc.sync.dma_start(out=dbg_kt[:, :, :], in_=kt_sb)
            nc.sync.dma_start(out=dbg_vone[:, :, :, :], in_=vone)
            nc.sync.dma_start(out=dbg_qt[:, :, :], in_=qt_sb)
            nc.sync.dma_start(out=dbg_at[:, :, :], in_=at_sb)

        # ---- phase 4: output projection (own seq quarter, full output) --
        with (
            tc.tile_pool(name="wo_pool", bufs=3) as wo_pool,
            tc.tile_pool(name="y_pool", bufs=2) as y_pool,
            tc.tile_pool(name="y_ps", bufs=3, space="PSUM") as y_ps,
        ):
            for m in range(16):
                wot = wo_pool.tile([128, _KC, 128], F16, tag="wot")
                nc.sync.dma_start(out=wot, in_=wo_d[m, :, :, :])
                psy = y_ps.tile([128, _SQ], F32, tag="psy")
                for kc in range(_KC):
                    nc.tensor.matmul(
                        psy,
                        wot[:, kc, :],
                        at_sb[:, kc, :],
                        start=kc == 0,
                        stop=kc == _KC - 1,
                    )
                y_sb = y_pool.tile([128, _SQ], F32, tag="y")
                nc.scalar.copy(y_sb, psy)
                nc.sync.dma_start(out=yt_d[m * 128 : (m + 1) * 128, :], in_=y_sb)

    nc.compile()
    return nc


def kernel(X, Wq, Wk, Wv, Wo):
    global _nc_cache, _last_results
    X = np.asarray(X, dtype=np.float32)
    Wq = np.asarray(Wq, dtype=np.float32)
    Wk = np.asarray(Wk, dtype=np.float32)
    Wv = np.asarray(Wv, dtype=np.float32)
    Wo = np.asarray(Wo, dtype=np.float32)

    if _nc_cache is None:
        _nc_cache = _build()
    nc = _nc_cache

    # head-pair-unit permutation: unit u -> q heads (u, u+16)
    qcols = []
    for u in range(_NU):
        qcols.extend(range(u * _HD, (u + 1) * _HD))
        qcols.extend(range((u + 16) * _HD, (u + 17) * _HD))
    # kv-pair blocks: gg -> kv heads (gg, gg+4)
    kvcols = []
    for gg in range(4):
        kvcols.extend(range(gg * _HD, (gg + 1) * _HD))
        kvcols.extend(range((gg + 4) * _HD, (gg + 5) * _HD))

    wq_p = np.ascontiguousarray(
        Wq.astype(np.float16)[:, qcols]
        .reshape(_KC, 128, _NU, 128)
        .transpose(2, 1, 0, 3)
    )
    wk_p = np.ascontiguousarray(
        Wk.astype(np.float16)[:, kvcols].reshape(_KC, 128, 512).transpose(1, 0, 2)
    )
    wv_p = np.ascontiguousarray(
        Wv.astype(np.float16)[:, kvcols].reshape(_KC, 128, 512).transpose(1, 0, 2)
    )
    wo_p = np.ascontiguousarray(
        Wo.astype(np.float16)[qcols, :]
        .reshape(_KC, 128, 16, 128)
        .transpose(2, 1, 0, 3)
    )

    X16 = X.astype(np.float16)
    in_maps = []
    for c in range(_NCORES):
        b, j = divmod(c, 4)
        xt_q = np.ascontiguousarray(
            X16[b].T[:, j * _SQ : (j + 1) * _SQ]
            .reshape(_KC, 128, _SQ)
            .transpose(1, 0, 2)
        )
        in_maps.append(
            {"xt": xt_q, "wq": wq_p, "wk": wk_p, "wv": wv_p, "wo": wo_p}
        )

    trace = bool(os.environ.get("KERNEL_TRACE"))
    res = run_bass_kernel_spmd(
        nc, in_maps, core_ids=list(range(_NCORES)), trace=trace
    )
    _last_results = res

    Y = np.empty((_BATCH, _SEQ, _HIDDEN), dtype=np.float32)
    for c in range(_NCORES):
        b, j = divmod(c, 4)
        Y[b][j * _SQ : (j + 1) * _SQ, :] = res.results[c]["yt"].T
    return Y
